# revision 5
# baseline (speedup 1.0000x reference)
"""Chamfer distance kernel for Trainium2, 8 NeuronCores.

Problem: B=4 batches, N=M=8192 points, C=3 coords.
  dist1[b,n] = min_m ||xyz1[b,n]-xyz2[b,m]||^2 ;  dist2[b,m] = min_n ||...||^2

Sharding: 4 batches x 2 directions = 8 perfectly balanced SPMD core tasks.
Each core solves one (query=A[8192], database=B[8192]) brute-force pass.

Per-core algorithm:
  dist[n] = min_m ( sq1[n] + sq2[m] - 2*x[n].y[m] )
The full nonnegative distance is computed on the TensorEngine as a K=18
bf16 matmul (hi/lo bf16 splits of coordinates and 3-way splits of both
squared norms => ~fp32 accuracy at full PE speed) into PSUM.  Keeping
sq1 inside the matmul keeps PSUM values small near the minima, so the
f16 SBUF staging below loses no meaningful precision.

The row-min is the bottleneck: every PSUM element must be first-touched
by exactly one of the two engines that can read PSUM:
  - DVE  scalar_tensor_tensor  min(psum_unit, f16_partner) -> f16
    (1.04 ns/elem, simultaneously *disposing* of a converted partner)
  - ACT  copy psum -> SBUF f16 (0.83 ns/elem)
Per row-tile of 128 queries x 8192 db points (8 units of [128,1024]),
delta units go to DVE and 8-delta to ACT; surplus converted tiles are
folded by DVE STT ops in all-SBUF f16 4x mode (0.26 ns/elem) and the
final [128,1024] f16 tile is folded 1024->128 then tensor_reduce'd into
the output column.  delta cycles 3,3,3,2 to balance DVE vs ACT load.

Feature matrices are built wide ([128, 64*128] bf16, one 128x128
transpose chunk per point so lhsT and rhs share partition base 0) and
transposed with PE-transposes + ScalarE/DVE copies.
"""

import sys
import copy

sys.path.insert(0, "/opt/trn_rl_repo")

import numpy as np

import bass_rust
import concourse.bass as bass
import concourse.tile as tile
from concourse import mybir
from concourse.bass_utils import run_bass_kernel_spmd

F32 = mybir.dt.float32
F16 = mybir.dt.float16
BF16 = mybir.dt.bfloat16

N = 8192          # query points per core
M = 8192          # database points per core
P = 128           # partitions
Q = N // P        # 64 points per partition stripe (n = p*Q + k)
S = 128           # feature slots per point: one 128x128 transpose chunk per
                  # point, so every feature row lands at partition base 0
                  # (matmul requires lhsT and rhs to share base partition)
K = 18            # matmul contraction rows actually used
CHUNK = 512       # matmul free-dim chunk (one PSUM bank of fp32)
UNIT = 1024       # PSUM consumption unit (2 banks)
NU = M // UNIT    # 8 units per row-tile
NROWS = Q         # row-tiles to process (tunable for benchmarking only)

MN = mybir.AluOpType.min
BIG = 1e30


def _split_excess_waits(nc, max_waits=1):
    # This container's walrus codegen only supports a single sem-wait
    # command per instruction ("Too many sync wait commands"). Hoist excess
    # sem waits onto NoOps inserted just before the offender on the same
    # engine (program order preserves blocking semantics).
    n_split = 0
    for f in nc.m.functions:
        for b in f.blocks:
            insts = b.instructions
            for ins in list(insts):
                si = ins.sync_info
                if si is None:
                    continue
                w = list(si.on_wait)
                if len(w) <= max_waits:
                    continue
                idx = insts.index(ins)
                keep = w[-max_waits:]
                extra = w[:-max_waits]
                ins.sync_info = bass_rust.SyncInfo(
                    on_wait=keep, on_update=list(si.on_update)
                )
                for j, wt in enumerate(extra):
                    c = bass_rust.InstNoOp(name=f"{ins.name}-wsplit{j}", ins=[], outs=[])
                    c.engine = ins.engine
                    c.sync_info = bass_rust.SyncInfo(on_wait=[wt], on_update=[])
                    insts.insert(idx + j, c)
                    n_split += 1
    return n_split


def _split3(nc, pool, v, f3, s0, side):
    """3-way bf16 split of [P,Q] f32 `v` into slots f3[:, :, s0:s0+3]."""
    v_ = nc.vector
    v_.tensor_copy(f3[:, :, s0:s0 + 1], v[:].rearrange("p (k o) -> p k o", o=1))
    hf = pool.tile([P, Q], F32, tag=f"{side}_3s_hf{s0}")
    v_.tensor_copy(hf[:], f3[:, :, s0:s0 + 1])
    r1 = pool.tile([P, Q], F32, tag=f"{side}_3s_r1{s0}")
    v_.tensor_tensor(r1[:], v[:], hf[:], op=mybir.AluOpType.subtract)
    v_.tensor_copy(f3[:, :, s0 + 1:s0 + 2], r1[:].rearrange("p (k o) -> p k o", o=1))
    mf = pool.tile([P, Q], F32, tag=f"{side}_3s_mf{s0}")
    v_.tensor_copy(mf[:], f3[:, :, s0 + 1:s0 + 2])
    r2 = pool.tile([P, Q], F32, tag=f"{side}_3s_r2{s0}")
    v_.tensor_tensor(r2[:], r1[:], mf[:], op=mybir.AluOpType.subtract)
    v_.tensor_copy(f3[:, :, s0 + 2:s0 + 3], r2[:].rearrange("p (k o) -> p k o", o=1))


def _prep_side(nc, pool, xyz_dram, side):
    """DMA [8192,3] f32 -> wide layout, build bf16 feature slots.

    Row pairing (A row k multiplies B row k; sum over k = full distance):
      rows 0-2 : A sq1 h/m/l        B ones
      rows 3-5 : A ones             B sq2 h/m/l
      rows 6-8 : A xh               B -2yh
      rows 9-11: A xh               B -2yl
      rows 12-14: A xl              B -2yh
      rows 15-17: A xl              B -2yl
    Returns feat_wide [128, Q*S] bf16 (slots 18..127 left as garbage --
    they transpose into stA/stB partitions >= 18 which no matmul reads).
    """
    v = nc.vector

    w = pool.tile([P, Q * 3], F32, tag=f"{side}_w")
    nc.sync.dma_start(w[:], xyz_dram.rearrange("(p k) c -> p (k c)", p=P))

    feat = pool.tile([P, Q * S], BF16, tag=f"{side}_feat")
    f3 = feat[:].rearrange("p (k s) -> p k s", s=S)

    hi_b = pool.tile([P, Q * 3], BF16, tag=f"{side}_hib")
    v.tensor_copy(hi_b[:], w[:])                       # round to bf16
    hi_f = pool.tile([P, Q * 3], F32, tag=f"{side}_hif")
    v.tensor_copy(hi_f[:], hi_b[:])                    # exact back to f32
    lo_f = pool.tile([P, Q * 3], F32, tag=f"{side}_lof")
    v.tensor_tensor(lo_f[:], w[:], hi_f[:], op=mybir.AluOpType.subtract)
    lo_b = pool.tile([P, Q * 3], BF16, tag=f"{side}_lob")
    v.tensor_copy(lo_b[:], lo_f[:])                    # round residual to bf16
    lo_xf = pool.tile([P, Q * 3], F32, tag=f"{side}_loxf")
    v.tensor_copy(lo_xf[:], lo_b[:])                   # exact f32 of bf16 lo

    # x_hat = hi + lo  (exact in f32; <=18 mantissa bits)
    hat = pool.tile([P, Q * 3], F32, tag=f"{side}_hat")
    v.tensor_tensor(hat[:], hi_f[:], lo_xf[:], op=mybir.AluOpType.add)
    # sq = sum_c x_hat_c^2
    prod = pool.tile([P, Q * 3], F32, tag=f"{side}_prod")
    v.tensor_tensor(prod[:], hat[:], hat[:], op=mybir.AluOpType.mult)
    sq = pool.tile([P, Q], F32, tag=f"{side}_sq")
    v.tensor_reduce(
        sq[:],
        prod[:].rearrange("p (k c) -> p k c", c=3),
        axis=mybir.AxisListType.X,
        op=mybir.AluOpType.add,
    )

    hi3 = hi_b[:].rearrange("p (k c) -> p k c", c=3)
    lo3 = lo_b[:].rearrange("p (k c) -> p k c", c=3)
    if side == "a":
        _split3(nc, pool, sq, f3, 0, side)             # rows 0-2: sq1 splits
        v.memset(f3[:, :, 3:6], 1.0)                   # rows 3-5: ones
        v.tensor_copy(f3[:, :, 6:9], hi3)
        v.tensor_copy(f3[:, :, 9:12], hi3)
        v.tensor_copy(f3[:, :, 12:15], lo3)
        v.tensor_copy(f3[:, :, 15:18], lo3)
    else:
        v.memset(f3[:, :, 0:3], 1.0)                   # rows 0-2: ones
        _split3(nc, pool, sq, f3, 3, side)             # rows 3-5: sq2 splits
        hi3f = hi_f[:].rearrange("p (k c) -> p k c", c=3)
        lo3f = lo_xf[:].rearrange("p (k c) -> p k c", c=3)
        v.tensor_scalar_mul(f3[:, :, 6:9], hi3f, -2.0)
        v.tensor_scalar_mul(f3[:, :, 12:15], hi3f, -2.0)
        v.tensor_scalar_mul(f3[:, :, 9:12], lo3f, -2.0)
        v.tensor_scalar_mul(f3[:, :, 15:18], lo3f, -2.0)
    return feat


def build_nc(repeat=1):
    import contextlib
    nc = bass.Bass()
    a_xyz = nc.dram_tensor("a_xyz", [N, 3], F32, kind="ExternalInput")
    b_xyz = nc.dram_tensor("b_xyz", [M, 3], F32, kind="ExternalInput")
    ident = nc.dram_tensor("ident", [P, P], BF16, kind="ExternalInput")
    out = nc.dram_tensor("dist", [N], F32, kind="ExternalOutput")

    with tile.TileContext(nc) as tc:
        with contextlib.ExitStack() as stack:
            if repeat > 1:
                stack.enter_context(tc.For_i(0, repeat, 1))
            prep = stack.enter_context(tc.tile_pool(name="prep", bufs=1))
            stage = stack.enter_context(tc.tile_pool(name="stage", bufs=1))
            res = stack.enter_context(tc.tile_pool(name="res", bufs=1))
            feat_a = _prep_side(nc, prep, a_xyz, "a")
            feat_b = _prep_side(nc, prep, b_xyz, "b")

            # Transpose wide features into matmul layout: PE transpose
            # (idle TensorE) into [128,512] bf16 PSUM tiles (4 chunks each),
            # copied out alternately by ScalarE and DVE.
            id_t = stage.tile([P, P], BF16, tag="ident")
            nc.sync.dma_start(id_t[:], ident[:])
            stA = stage.tile([P, Q * S], BF16, tag="stA")
            stB = stage.tile([P, Q * S], BF16, tag="stB")
            n_chunks = (Q * S) // P                     # 64 per side
            with tc.tile_pool(name="tpsum", bufs=4, space="PSUM") as tpp:
                for src, dst in ((feat_b, stB), (feat_a, stA)):
                    for t4 in range(n_chunks // 4):
                        tp = tpp.tile([P, 4 * P], BF16, tag="tps")
                        for j in range(4):
                            t = t4 * 4 + j
                            nc.tensor.transpose(
                                tp[:, j * P:(j + 1) * P],
                                src[:, t * P:(t + 1) * P], id_t[:])
                        sl = slice(t4 * 4 * P, (t4 + 1) * 4 * P)
                        if t4 % 2 == 0:
                            nc.scalar.copy(dst[:, sl], tp[:])
                        else:
                            nc.vector.tensor_copy(dst[:, sl], tp[:])

            pp = stack.enter_context(tc.tile_pool(name="psum", bufs=4, space="PSUM"))
            cpool = stack.enter_context(tc.tile_pool(name="cpool", bufs=14))
            tpool = stack.enter_context(tc.tile_pool(name="tpool", bufs=8))
            fpool = stack.enter_context(tc.tile_pool(name="fpool", bufs=6))

            dist = res.tile([P, Q], F32, tag="dist")

            def stt_min(out_ap, in0_ap, in1_ap):
                nc.vector.scalar_tensor_tensor(
                    out_ap, in0_ap, BIG, in1_ap, op0=MN, op1=MN)

            # Per-row-tile unit routing: delta units first-touched by DVE
            # (paired with a just-converted partner), the rest by ACT.
            # Pattern arrays: 'a' = ACT convert, 'd' = DVE STT.
            ROUTE = {3: "adadadaa", 2: "adadaaaa"}

            def do_row(r):
                delta = 2 if (r % 4) == 3 else 3
                route = ROUTE[delta]
                lhsT = stA[0:K, r * P:(r + 1) * P]
                conv = []          # ACT-converted f16 tiles, in order
                ts = []            # DVE L0 outputs
                for u in range(NU):
                    ps = pp.tile([P, UNIT], F32, tag="ps")
                    for c in range(UNIT // CHUNK):
                        cc = u * UNIT + c * CHUNK
                        nc.tensor.matmul(
                            ps[:, c * CHUNK:(c + 1) * CHUNK],
                            lhsT, stB[0:K, cc:cc + CHUNK],
                            start=True, stop=True,
                        )
                    if route[u] == "a":
                        ctile = cpool.tile([P, UNIT], F16, tag="c")
                        nc.scalar.copy(ctile[:], ps[:])
                        conv.append(ctile)
                    else:
                        t = tpool.tile([P, UNIT], F16, tag="t")
                        stt_min(t[:], ps[:], conv[len(ts)][:])
                        ts.append(t)
                # tiles still alive: ts + conv[delta:] (surplus conversions)
                return ts + conv[delta:]

            def fold_row(r, tiles):
                # binary-fold the row's live tiles to one, then 1024->128
                # fold + reduce into the output column. All in f16 SBUF 4x.
                while len(tiles) > 1:
                    nt = []
                    for i in range(0, len(tiles) - 1, 2):
                        m = fpool.tile([P, UNIT], F16, tag="m")
                        stt_min(m[:], tiles[i][:], tiles[i + 1][:])
                        nt.append(m)
                    if len(tiles) % 2:
                        nt.append(tiles[-1])
                    tiles = nt
                mfin = tiles[0]
                s1 = fpool.tile([P, UNIT // 2], F16, tag="s1")
                stt_min(s1[:], mfin[:, 0:UNIT // 2], mfin[:, UNIT // 2:])
                s2 = fpool.tile([P, UNIT // 4], F16, tag="s2")
                stt_min(s2[:], s1[:, 0:UNIT // 4], s1[:, UNIT // 4:])
                s3 = fpool.tile([P, UNIT // 8], F16, tag="s3")
                stt_min(s3[:], s2[:, 0:UNIT // 8], s2[:, UNIT // 8:])
                nc.vector.tensor_reduce(
                    dist[:, r:r + 1], s3[:], axis=mybir.AxisListType.X, op=MN)

            # Software-pipeline: emit row r's PSUM stage, then row r-1's
            # fold/merge tail (so DVE never stalls waiting on late tiles).
            pend = None
            for r in range(NROWS):
                tiles = do_row(r)
                if pend is not None:
                    fold_row(r - 1, pend)
                pend = tiles
            if pend is not None:
                fold_row(NROWS - 1, pend)

            nc.sync.dma_start(out.rearrange("(p k) -> p k", p=P), dist[:])

    _split_excess_waits(nc)
    return nc


_NC_CACHE = {}


def _get_nc(repeat=1):
    if repeat not in _NC_CACHE:
        _NC_CACHE[repeat] = build_nc(repeat)
    return _NC_CACHE[repeat]


def kernel(xyz1, xyz2, _trace=False, _repeat=1):
    xyz1 = np.ascontiguousarray(np.asarray(xyz1, dtype=np.float32))
    xyz2 = np.ascontiguousarray(np.asarray(xyz2, dtype=np.float32))
    B = xyz1.shape[0]
    assert xyz1.shape == (B, N, 3) and xyz2.shape == (B, M, 3)

    nc = _get_nc(_repeat)
    import ml_dtypes
    ident = np.eye(P, dtype=ml_dtypes.bfloat16)
    in_maps = []
    for c in range(2 * B):
        b, d = c % B, c // B
        if d == 0:
            in_maps.append({"a_xyz": xyz1[b], "b_xyz": xyz2[b], "ident": ident})
        else:
            in_maps.append({"a_xyz": xyz2[b], "b_xyz": xyz1[b], "ident": ident})

    res = run_bass_kernel_spmd(
        nc, in_maps, core_ids=list(range(2 * B)), trace=_trace
    )
    dist1 = np.stack([res.results[b]["dist"] for b in range(B)])
    dist2 = np.stack([res.results[B + b]["dist"] for b in range(B)])
    if _trace:
        return (dist1, dist2), res
    return dist1, dist2


# revision 6
# speedup vs baseline: 1.5016x; 1.5016x over previous
"""Chamfer distance kernel for Trainium2, 8 NeuronCores.

Problem: B=4 batches, N=M=8192 points, C=3 coords.
  dist1[b,n] = min_m ||xyz1[b,n]-xyz2[b,m]||^2 ;  dist2[b,m] = min_n ||...||^2

Sharding: 4 batches x 2 directions = 8 perfectly balanced SPMD core tasks.
Each core solves one (query=A[8192], database=B[8192]) brute-force pass.

Per-core algorithm:
  dist[n] = min_m ( sq1[n] + sq2[m] - 2*x[n].y[m] )
The full nonnegative distance is computed on the TensorEngine as a K=18
bf16 matmul (hi/lo bf16 splits of coordinates and 3-way splits of both
squared norms => ~fp32 accuracy at full PE speed) into PSUM.  Keeping
sq1 inside the matmul keeps PSUM values small near the minima, so the
f16 SBUF staging below loses no meaningful precision.

The row-min is the bottleneck: every PSUM element must be first-touched
by exactly one of the two engines that can read PSUM:
  - DVE  scalar_tensor_tensor  min(psum_unit, f16_partner) -> f16
    (1.04 ns/elem, simultaneously *disposing* of a converted partner)
  - ACT  copy psum -> SBUF f16 (0.83 ns/elem)
Per row-tile of 128 queries x 8192 db points (8 units of [128,1024]),
delta units go to DVE and 8-delta to ACT; surplus converted tiles are
folded by DVE STT ops in all-SBUF f16 4x mode (0.26 ns/elem) and the
final [128,1024] f16 tile is folded 1024->128 then tensor_reduce'd into
the output column.  delta cycles 3,3,3,2 to balance DVE vs ACT load.

Feature matrices are built wide ([128, 64*128] bf16, one 128x128
transpose chunk per point so lhsT and rhs share partition base 0) and
transposed with PE-transposes + ScalarE/DVE copies.
"""

import sys
import copy

sys.path.insert(0, "/opt/trn_rl_repo")

import numpy as np

import bass_rust
import concourse.bass as bass
import concourse.tile as tile
from concourse import mybir
from concourse.bass_utils import run_bass_kernel_spmd

F32 = mybir.dt.float32
F16 = mybir.dt.float16
BF16 = mybir.dt.bfloat16

N = 8192          # query points per core
M = 8192          # database points per core
P = 128           # partitions
Q = N // P        # 64 points per partition stripe (n = p*Q + k)
S = 128           # feature slots per point: one 128x128 transpose chunk per
                  # point, so every feature row lands at partition base 0
                  # (matmul requires lhsT and rhs to share base partition)
K = 18            # matmul contraction rows actually used
CHUNK = 512       # matmul free-dim chunk (one PSUM bank of fp32)
UNIT = 1024       # PSUM consumption unit (2 banks)
NU = M // UNIT    # 8 units per row-tile
NROWS = Q         # row-tiles to process (tunable for benchmarking only)

MN = mybir.AluOpType.min
BIG = 1e30


def _split_excess_waits(nc, max_waits=1):
    # This container's walrus codegen only supports a single sem-wait
    # command per instruction ("Too many sync wait commands"). Hoist excess
    # sem waits onto NoOps inserted just before the offender on the same
    # engine (program order preserves blocking semantics).
    n_split = 0
    for f in nc.m.functions:
        for b in f.blocks:
            insts = b.instructions
            for ins in list(insts):
                si = ins.sync_info
                if si is None:
                    continue
                w = list(si.on_wait)
                if len(w) <= max_waits:
                    continue
                idx = insts.index(ins)
                keep = w[-max_waits:]
                extra = w[:-max_waits]
                ins.sync_info = bass_rust.SyncInfo(
                    on_wait=keep, on_update=list(si.on_update)
                )
                for j, wt in enumerate(extra):
                    c = bass_rust.InstNoOp(name=f"{ins.name}-wsplit{j}", ins=[], outs=[])
                    c.engine = ins.engine
                    c.sync_info = bass_rust.SyncInfo(on_wait=[wt], on_update=[])
                    insts.insert(idx + j, c)
                    n_split += 1
    return n_split


def _split3(nc, pool, v, f3, s0, side):
    """3-way bf16 split of [P,Q] f32 `v` into slots f3[:, :, s0:s0+3]."""
    v_ = nc.vector
    v_.tensor_copy(f3[:, :, s0:s0 + 1], v[:].rearrange("p (k o) -> p k o", o=1))
    hf = pool.tile([P, Q], F32, tag=f"{side}_3s_hf{s0}")
    v_.tensor_copy(hf[:], f3[:, :, s0:s0 + 1])
    r1 = pool.tile([P, Q], F32, tag=f"{side}_3s_r1{s0}")
    v_.tensor_tensor(r1[:], v[:], hf[:], op=mybir.AluOpType.subtract)
    v_.tensor_copy(f3[:, :, s0 + 1:s0 + 2], r1[:].rearrange("p (k o) -> p k o", o=1))
    mf = pool.tile([P, Q], F32, tag=f"{side}_3s_mf{s0}")
    v_.tensor_copy(mf[:], f3[:, :, s0 + 1:s0 + 2])
    r2 = pool.tile([P, Q], F32, tag=f"{side}_3s_r2{s0}")
    v_.tensor_tensor(r2[:], r1[:], mf[:], op=mybir.AluOpType.subtract)
    v_.tensor_copy(f3[:, :, s0 + 2:s0 + 3], r2[:].rearrange("p (k o) -> p k o", o=1))


def _prep_side(nc, pool, xyz_dram, side):
    """DMA [8192,3] f32 -> wide layout, build bf16 feature slots.

    Row pairing (A row k multiplies B row k; sum over k = full distance):
      rows 0-2 : A sq1 h/m/l        B ones
      rows 3-5 : A ones             B sq2 h/m/l
      rows 6-8 : A xh               B -2yh
      rows 9-11: A xh               B -2yl
      rows 12-14: A xl              B -2yh
      rows 15-17: A xl              B -2yl
    Returns feat_wide [128, Q*S] bf16 (slots 18..127 left as garbage --
    they transpose into stA/stB partitions >= 18 which no matmul reads).
    """
    v = nc.vector

    w = pool.tile([P, Q * 3], F32, tag=f"{side}_w")
    nc.sync.dma_start(w[:], xyz_dram.rearrange("(p k) c -> p (k c)", p=P))

    feat = pool.tile([P, Q * S], BF16, tag=f"{side}_feat")
    f3 = feat[:].rearrange("p (k s) -> p k s", s=S)

    hi_b = pool.tile([P, Q * 3], BF16, tag=f"{side}_hib")
    v.tensor_copy(hi_b[:], w[:])                       # round to bf16
    hi_f = pool.tile([P, Q * 3], F32, tag=f"{side}_hif")
    v.tensor_copy(hi_f[:], hi_b[:])                    # exact back to f32
    lo_f = pool.tile([P, Q * 3], F32, tag=f"{side}_lof")
    v.tensor_tensor(lo_f[:], w[:], hi_f[:], op=mybir.AluOpType.subtract)
    lo_b = pool.tile([P, Q * 3], BF16, tag=f"{side}_lob")
    v.tensor_copy(lo_b[:], lo_f[:])                    # round residual to bf16
    lo_xf = pool.tile([P, Q * 3], F32, tag=f"{side}_loxf")
    v.tensor_copy(lo_xf[:], lo_b[:])                   # exact f32 of bf16 lo

    # x_hat = hi + lo  (exact in f32; <=18 mantissa bits)
    hat = pool.tile([P, Q * 3], F32, tag=f"{side}_hat")
    v.tensor_tensor(hat[:], hi_f[:], lo_xf[:], op=mybir.AluOpType.add)
    # sq = sum_c x_hat_c^2
    prod = pool.tile([P, Q * 3], F32, tag=f"{side}_prod")
    v.tensor_tensor(prod[:], hat[:], hat[:], op=mybir.AluOpType.mult)
    sq = pool.tile([P, Q], F32, tag=f"{side}_sq")
    v.tensor_reduce(
        sq[:],
        prod[:].rearrange("p (k c) -> p k c", c=3),
        axis=mybir.AxisListType.X,
        op=mybir.AluOpType.add,
    )

    hi3 = hi_b[:].rearrange("p (k c) -> p k c", c=3)
    lo3 = lo_b[:].rearrange("p (k c) -> p k c", c=3)
    if side == "a":
        _split3(nc, pool, sq, f3, 0, side)             # rows 0-2: sq1 splits
        v.memset(f3[:, :, 3:6], 1.0)                   # rows 3-5: ones
        v.tensor_copy(f3[:, :, 6:9], hi3)
        v.tensor_copy(f3[:, :, 9:12], hi3)
        v.tensor_copy(f3[:, :, 12:15], lo3)
        v.tensor_copy(f3[:, :, 15:18], lo3)
    else:
        v.memset(f3[:, :, 0:3], 1.0)                   # rows 0-2: ones
        _split3(nc, pool, sq, f3, 3, side)             # rows 3-5: sq2 splits
        hi3f = hi_f[:].rearrange("p (k c) -> p k c", c=3)
        lo3f = lo_xf[:].rearrange("p (k c) -> p k c", c=3)
        v.tensor_scalar_mul(f3[:, :, 6:9], hi3f, -2.0)
        v.tensor_scalar_mul(f3[:, :, 12:15], hi3f, -2.0)
        v.tensor_scalar_mul(f3[:, :, 9:12], lo3f, -2.0)
        v.tensor_scalar_mul(f3[:, :, 15:18], lo3f, -2.0)
    return feat


def build_nc(repeat=1):
    import contextlib
    nc = bass.Bass()
    a_xyz = nc.dram_tensor("a_xyz", [N, 3], F32, kind="ExternalInput")
    b_xyz = nc.dram_tensor("b_xyz", [M, 3], F32, kind="ExternalInput")
    ident = nc.dram_tensor("ident", [P, P], BF16, kind="ExternalInput")
    out = nc.dram_tensor("dist", [N], F32, kind="ExternalOutput")

    with tile.TileContext(nc) as tc:
        with contextlib.ExitStack() as stack:
            if repeat > 1:
                stack.enter_context(tc.For_i(0, repeat, 1))
            prep = stack.enter_context(tc.tile_pool(name="prep", bufs=1))
            stage = stack.enter_context(tc.tile_pool(name="stage", bufs=1))
            res = stack.enter_context(tc.tile_pool(name="res", bufs=1))
            feat_a = _prep_side(nc, prep, a_xyz, "a")
            feat_b = _prep_side(nc, prep, b_xyz, "b")

            # Transpose wide features into matmul layout: PE transpose
            # (idle TensorE) into [128,512] bf16 PSUM tiles (4 chunks each),
            # copied out alternately by ScalarE and DVE.
            id_t = stage.tile([P, P], BF16, tag="ident")
            nc.sync.dma_start(id_t[:], ident[:])
            stA = stage.tile([P, Q * S], BF16, tag="stA")
            stB = stage.tile([P, Q * S], BF16, tag="stB")
            n_chunks = (Q * S) // P                     # 64 per side
            with tc.tile_pool(name="tpsum", bufs=4, space="PSUM") as tpp:
                for src, dst in ((feat_b, stB), (feat_a, stA)):
                    for t4 in range(n_chunks // 4):
                        tp = tpp.tile([P, 4 * P], BF16, tag="tps")
                        for j in range(4):
                            t = t4 * 4 + j
                            nc.tensor.transpose(
                                tp[:, j * P:(j + 1) * P],
                                src[:, t * P:(t + 1) * P], id_t[:])
                        sl = slice(t4 * 4 * P, (t4 + 1) * 4 * P)
                        if t4 % 2 == 0:
                            nc.scalar.copy(dst[:, sl], tp[:])
                        else:
                            nc.vector.tensor_copy(dst[:, sl], tp[:])

            # Main loop.  ACT is the sole PSUM consumer: it converts each
            # [128,2048] PSUM group (4 banks) to an f16 chunk of a per-row
            # [128, 8192] SBUF buffer while the PE fills the other 4 banks.
            # DVE then needs a single full-row tensor_reduce(min) per
            # row-tile, fully decoupled from PSUM.
            GRP = 2048
            NG = M // GRP                               # 4 groups per row-tile
            pp = stack.enter_context(tc.tile_pool(name="psum", bufs=2, space="PSUM"))
            rpool = stack.enter_context(tc.tile_pool(name="rpool", bufs=3))

            dist = res.tile([P, Q], F32, tag="dist")

            for r in range(NROWS):
                lhsT = stA[0:K, r * P:(r + 1) * P]
                buf = rpool.tile([P, M], F16, tag="row16")
                for g in range(NG):
                    ps = pp.tile([P, GRP], F32, tag="ps")
                    for c in range(GRP // CHUNK):
                        cc = g * GRP + c * CHUNK
                        nc.tensor.matmul(
                            ps[:, c * CHUNK:(c + 1) * CHUNK],
                            lhsT, stB[0:K, cc:cc + CHUNK],
                            start=True, stop=True,
                        )
                    nc.scalar.copy(buf[:, g * GRP:(g + 1) * GRP], ps[:])
                nc.vector.tensor_reduce(
                    dist[:, r:r + 1], buf[:], axis=mybir.AxisListType.X, op=MN)

            nc.sync.dma_start(out.rearrange("(p k) -> p k", p=P), dist[:])

    _split_excess_waits(nc)
    return nc


_NC_CACHE = {}


def _get_nc(repeat=1):
    if repeat not in _NC_CACHE:
        _NC_CACHE[repeat] = build_nc(repeat)
    return _NC_CACHE[repeat]


def kernel(xyz1, xyz2, _trace=False, _repeat=1):
    xyz1 = np.ascontiguousarray(np.asarray(xyz1, dtype=np.float32))
    xyz2 = np.ascontiguousarray(np.asarray(xyz2, dtype=np.float32))
    B = xyz1.shape[0]
    assert xyz1.shape == (B, N, 3) and xyz2.shape == (B, M, 3)

    nc = _get_nc(_repeat)
    import ml_dtypes
    ident = np.eye(P, dtype=ml_dtypes.bfloat16)
    in_maps = []
    for c in range(2 * B):
        b, d = c % B, c // B
        if d == 0:
            in_maps.append({"a_xyz": xyz1[b], "b_xyz": xyz2[b], "ident": ident})
        else:
            in_maps.append({"a_xyz": xyz2[b], "b_xyz": xyz1[b], "ident": ident})

    res = run_bass_kernel_spmd(
        nc, in_maps, core_ids=list(range(2 * B)), trace=_trace
    )
    dist1 = np.stack([res.results[b]["dist"] for b in range(B)])
    dist2 = np.stack([res.results[B + b]["dist"] for b in range(B)])
    if _trace:
        return (dist1, dist2), res
    return dist1, dist2
